# revision 5
# baseline (speedup 1.0000x reference)
"""Trainium2 Bass kernel for nn_AttentionBlock2 (gnn_message_passing).

8 NeuronCores, SPMD, no collectives:
  - 2 batches x 4 cores; within a batch, nodes sorted by r-cell and split
    into 4 contiguous cell ranges (disjoint output slices per core).
  - v-gather: dma_gather (SWDGE, 4 queues, single_packet=False) of
    quad-packed bf16 rows (4 feature rows per 512B table row -> int16
    indexable), then an on-chip 4-way predicated select.
  - Math refactor: q' = (Wq^T Wk / sqrt(E)) r ; output proj Wov = Wo@Wv
    applied after the scatter.
  - r is never gathered per node: sequential range load -> PE q'-table ->
    per-node expansion matmuls vs one-hot tiles (DVE int16 compares).
  - scatter-add: PE matmuls of xbar vs one-hot match tiles, PSUM-accumulated
    per 256-cell output window. Collision-free by construction.
"""

import sys
import types
import numpy as np
import ml_dtypes

B = 2
CV = 64
CR = 20
E = 64
CO = 64
BUNDLE = 4
P = 128
PER_B = 4
SG = 1024            # nodes per dma_gather call (SG*4 = 4096 idx)
GRP = 512            # nodes per compute group
WCT = 256            # scatter window width (cells)
WCW = 128            # q'-expansion window width (cells)
NEG = -(10 ** 9)

BF16 = ml_dtypes.bfloat16


def _plan(v2p, r2p):
    """Data-dependent but core-uniform plan."""
    Nn = r2p.shape[1]
    cores = []
    for b in range(B):
        cells = r2p[b, :, 0].astype(np.int64)
        order = np.argsort(cells, kind="stable")
        sc = cells[order]
        bounds = []
        for k in range(1, PER_B):
            c = sc[k * Nn // PER_B]
            bounds.append(int(np.searchsorted(sc, c)))
        pb = [0] + bounds + [Nn]
        for pi in range(PER_B):
            lo, hi = pb[pi], pb[pi + 1]
            nodes = order[lo:hi]
            clo = int(sc[lo])
            cores.append(dict(b=b, nodes=nodes, clo=clo,
                              width=int(sc[hi - 1]) + 1 - clo))
    nmax = max(len(c["nodes"]) for c in cores)
    NGRP = -(-nmax // GRP)
    gps = SG // GRP
    if NGRP % gps:
        NGRP += gps - NGRP % gps
    NN = NGRP * GRP
    NCHUNKS = NN // P
    NSGS = NN // SG
    wmax = max(c["width"] for c in cores)
    CT = -(-wmax // WCT)
    W_OUT = CT * WCT
    CTT = W_OUT // WCW

    for c in cores:
        n = len(c["nodes"])
        c["n"] = n
        cr = np.full(NN, NEG, np.int64)
        cr[:n] = r2p[c["b"], c["nodes"], 0].astype(np.int64) - c["clo"]
        c["cell"] = cr
        vr = np.zeros((NN, BUNDLE), np.int64)
        vr[:n] = v2p[c["b"], :, 0].reshape(Nn, BUNDLE)[c["nodes"]]
        c["vrow"] = vr

    ch_lo_s = np.full(CT, 10 ** 9, np.int64)
    ch_hi_s = np.zeros(CT, np.int64)
    ch_lo_t = np.full(CTT, 10 ** 9, np.int64)
    ch_hi_t = np.zeros(CTT, np.int64)
    for c in cores:
        cr = c["cell"]
        valid = cr > NEG
        for W, lo_arr, hi_arr, CN in ((WCT, ch_lo_s, ch_hi_s, CT),
                                      (WCW, ch_lo_t, ch_hi_t, CTT)):
            w_of = np.where(valid, cr // W, -1)
            for wi in range(CN):
                idx = np.nonzero(w_of == wi)[0]
                if len(idx):
                    lo_arr[wi] = min(lo_arr[wi], idx[0] // P)
                    hi_arr[wi] = max(hi_arr[wi], idx[-1] // P + 1)
    ch_lo_s = np.where(ch_lo_s > ch_hi_s, 0, ch_lo_s)
    nwin_s = np.maximum(ch_hi_s - ch_lo_s, 1).astype(np.int64)
    ch_lo_t = np.where(ch_lo_t > ch_hi_t, 0, ch_lo_t)
    nwin_t = np.maximum(ch_hi_t - ch_lo_t, 1).astype(np.int64)

    cover = [[] for _ in range(NCHUNKS)]
    for cw in range(CTT):
        for ch in range(int(ch_lo_t[cw]), int(ch_lo_t[cw] + nwin_t[cw])):
            if 0 <= ch < NCHUNKS:
                cover[ch].append(cw)
    for ch in range(NCHUNKS):
        if not cover[ch]:
            cover[ch].append(0)

    return dict(cores=cores, NN=NN, NGRP=NGRP, NCHUNKS=NCHUNKS, NSGS=NSGS,
                CT=CT, W_OUT=W_OUT, CTT=CTT,
                ch_lo_s=ch_lo_s, nwin_s=nwin_s, cover=cover)


def _core_arrays(c, plan, v_feat, r_feat):
    NN, NSGS, CT, W_OUT = plan["NN"], plan["NSGS"], plan["CT"], plan["W_OUT"]
    NGRP, NCHUNKS = plan["NGRP"], plan["NCHUNKS"]
    b = c["b"]
    out = {}
    vt = np.ascontiguousarray(v_feat[b].T).astype(BF16)       # [Mv, 64]
    out["vtab4"] = np.ascontiguousarray(vt.reshape(-1, BUNDLE * CV))
    rt = np.zeros((W_OUT, CR), np.float32)
    w = min(c["width"], W_OUT)
    rt[:w] = r_feat[b].T[c["clo"]: c["clo"] + w]
    out["rtabs"] = rt

    vr = c["vrow"]
    NIDX = SG * BUNDLE
    gps = SG // GRP
    vidx = np.zeros((NSGS, NIDX), np.int64)
    quad = np.zeros((NSGS, P, gps * 16), np.uint8)
    ar = np.arange(P)
    for sg in range(NSGS):
        for gs in range(gps):
            for j in range(BUNDLE):
                for t in range(4):
                    k = gs * 16 + j * 4 + t
                    nodes = sg * SG + gs * GRP + t * P + ar
                    rows = vr[nodes, j]
                    vidx[sg, k * P + ar] = rows // 4
                    quad[sg, :, k] = rows % 4
    assert vidx.max() < 32768, "v row index exceeds int16 quad range"
    out["vidx"] = np.ascontiguousarray(
        np.tile(vidx.reshape(NSGS, NIDX // 16, 16).transpose(0, 2, 1),
                (1, 8, 1))).astype(np.int16)

    qm = np.zeros((NGRP, 3, P, 16), np.float32)
    for g in range(NGRP):
        sg, gs = g // gps, g % gps
        qq = quad[sg, :, gs * 16:(gs + 1) * 16]
        for qi in (1, 2, 3):
            qm[g, qi - 1] = (qq == qi)
    out["qmask"] = qm.astype(np.uint8)

    cr16 = np.clip(c["cell"], -32768, 32767).astype(np.int16)
    out["cellrep"] = np.ascontiguousarray(np.broadcast_to(cr16, (P, NN)))

    ch_lo_s, nwin_s = plan["ch_lo_s"], plan["nwin_s"]
    NWIN = int(nwin_s.sum())
    cc = np.full((NWIN, P), float(NEG), np.float32)
    wi = 0
    for ct in range(CT):
        for wv in range(int(nwin_s[ct])):
            ch = int(ch_lo_s[ct]) + wv
            if ch < NCHUNKS:
                vals = c["cell"][ch * P:(ch + 1) * P].astype(np.float64)
                vals = vals - ct * WCT
                vals[vals < -1e6] = NEG
                cc[wi] = vals.astype(np.float32)
            wi += 1
    out["cellcols"] = np.ascontiguousarray(cc.T)
    return out


def _build(plan, Mv):
    import concourse.bacc as bacc
    import concourse.mybir as mybir
    from concourse.tile import TileContext
    from concourse.masks import make_identity

    NN, NGRP = plan["NN"], plan["NGRP"]
    NCHUNKS, NSGS = plan["NCHUNKS"], plan["NSGS"]
    CT, W_OUT, CTT = plan["CT"], plan["W_OUT"], plan["CTT"]
    ch_lo_s, nwin_s, cover = plan["ch_lo_s"], plan["nwin_s"], plan["cover"]
    NWIN = int(nwin_s.sum())
    NIDX = SG * BUNDLE
    GPS = SG // GRP

    nc = bacc.Bacc("TRN2", target_bir_lowering=False, debug=False,
                   num_swdge_queues=4)
    dt = mybir.dt
    AL = mybir.AluOpType
    vtab4 = nc.declare_dram_parameter("vtab4", [Mv // 4, BUNDLE * CV], dt.bfloat16, isOutput=False)
    rtabs = nc.declare_dram_parameter("rtabs", [W_OUT, CR], dt.float32, isOutput=False)
    vidx_d = nc.declare_dram_parameter("vidx", [NSGS, P, NIDX // 16], dt.int16, isOutput=False)
    qmask_d = nc.declare_dram_parameter("qmask", [NGRP, 3, P, 16], dt.uint8, isOutput=False)
    cellrep_d = nc.declare_dram_parameter("cellrep", [P, NN], dt.int16, isOutput=False)
    cellcols_d = nc.declare_dram_parameter("cellcols", [P, NWIN], dt.float32, isOutput=False)
    a16_d = nc.declare_dram_parameter("a16", [CR, E], dt.bfloat16, isOutput=False)
    wov_d = nc.declare_dram_parameter("wovT", [E, CO], dt.bfloat16, isOutput=False)
    out_d = nc.declare_dram_parameter("out", [CO, W_OUT], dt.float32, isOutput=True)

    with TileContext(nc) as tc:
        with (
            tc.tile_pool(name="res", bufs=1) as res,
            tc.tile_pool(name="x4p", bufs=3) as x4p,
            tc.tile_pool(name="xp", bufs=3) as xp,
            tc.tile_pool(name="small", bufs=4) as small,
            tc.tile_pool(name="qgp", bufs=3) as qgp,
            tc.tile_pool(name="scp", bufs=3) as scp,
            tc.tile_pool(name="psA", bufs=2, space="PSUM") as psA,
            tc.tile_pool(name="psB", bufs=2, space="PSUM") as psB,
            tc.tile_pool(name="psC", bufs=2, space="PSUM") as psC,
            tc.tile_pool(name="psD", bufs=2, space="PSUM") as psD,
        ):
            # ---------- resident loads / constants ----------
            cellrep = res.tile([P, NN], dt.int16)
            nc.sync.dma_start(out=cellrep[:], in_=cellrep_d[:])
            cellcols = res.tile([P, NWIN], dt.float32)
            nc.sync.dma_start(out=cellcols[:], in_=cellcols_d[:])
            qmask = res.tile([P, NGRP, 3, 16], dt.uint8)
            nc.sync.dma_start(out=qmask[:],
                              in_=qmask_d[:].rearrange("g q p s -> p g q s"))
            rt = res.tile([P, CTT, CR], dt.float32)
            nc.sync.dma_start(out=rt[:],
                              in_=rtabs[:].rearrange("(cw p) f -> p cw f", p=P))
            a16 = res.tile([CR, E], dt.bfloat16)
            nc.sync.dma_start(out=a16[:], in_=a16_d[:])
            wovT = res.tile([E, CO], dt.bfloat16)
            nc.sync.dma_start(out=wovT[:], in_=wov_d[:])
            ident = res.tile([P, P], dt.float32)
            make_identity(nc, ident[:])
            iota256 = res.tile([P, WCT], dt.float32)
            nc.gpsimd.iota(iota256[:], pattern=[[1, WCT]], base=0,
                           channel_multiplier=0,
                           allow_small_or_imprecise_dtypes=True)
            iotaW = res.tile([P, CTT], dt.int16)
            nc.gpsimd.iota(iotaW[:], pattern=[[WCW, CTT]], base=0,
                           channel_multiplier=1)
            xbar = res.tile([P, NCHUNKS, E], dt.bfloat16)
            qtable = res.tile([P, CTT, E], dt.bfloat16)

            # ---------- q'-table ----------
            for cw in range(CTT):
                rT = psA.tile([CR, P], dt.float32, tag="psA")
                nc.tensor.transpose(out=rT[:], in_=rt[:, cw, :],
                                    identity=ident[:])
                rfm = small.tile([CR, P], dt.bfloat16, tag="rfm")
                nc.scalar.copy(out=rfm[:], in_=rT[:])
                qp = psB.tile([P, E], dt.float32, tag="psB")
                nc.tensor.matmul(out=qp[:], lhsT=rfm[:], rhs=a16[:],
                                 start=True, stop=True)
                nc.scalar.copy(out=qtable[:, cw, :], in_=qp[:])

            # ---------- per-supergroup: gather, select, attention ----------
            for sg in range(NSGS):
                vix = small.tile([P, NIDX // 16], dt.int16, tag="vix")
                nc.sync.dma_start(out=vix[:], in_=vidx_d[sg])
                x4 = x4p.tile([P, GPS * 16, BUNDLE * CV], dt.bfloat16, tag="x4")
                nc.gpsimd.dma_gather(
                    out_ap=x4[:], in_ap=vtab4[:], idxs_ap=vix[:],
                    num_idxs=NIDX, num_idxs_reg=NIDX, elem_size=BUNDLE * CV,
                    single_packet=False, queue_num=sg % 4)
                for gs in range(GPS):
                    g = sg * GPS + gs
                    x = xp.tile([P, 16, CV + 2], dt.bfloat16, tag="x")
                    xv = x[:, :, 0:CV]
                    x4g = x4[:, gs * 16:(gs + 1) * 16, :]
                    nc.vector.tensor_copy(out=xv, in_=x4g[:, :, 0:CV])
                    for qi in (1, 2, 3):
                        mk = qmask[:, g, qi - 1, :]
                        nc.vector.copy_predicated(
                            out=xv,
                            mask=mk[:, :, None].to_broadcast([P, 16, CV]),
                            data=x4g[:, :, qi * CV:(qi + 1) * CV])

                    qg = qgp.tile([P, 4, E], dt.bfloat16, tag="qg")
                    for t in range(4):
                        ch = g * 4 + t
                        qps = psB.tile([P, E], dt.float32, tag="psB")
                        cvr = cover[ch]
                        for ci, cw in enumerate(cvr):
                            mt = small.tile([P, P], dt.bfloat16, tag="mt")
                            nc.vector.tensor_tensor(
                                out=mt[:],
                                in0=iotaW[:, cw:cw + 1].to_broadcast([P, P]),
                                in1=cellrep[:, ch * P:(ch + 1) * P],
                                op=AL.is_equal)
                            nc.tensor.matmul(out=qps[:], lhsT=mt[:],
                                             rhs=qtable[:, cw, :],
                                             start=(ci == 0),
                                             stop=(ci == len(cvr) - 1))
                        nc.scalar.copy(out=qg[:, t, :], in_=qps[:])

                    prod = scp.tile([P, 16, CV], dt.float32, tag="prod")
                    nc.vector.tensor_tensor(
                        out=prod[:].rearrange("p (j t) f -> p j t f", j=4),
                        in0=xv.rearrange("p (j t) f -> p j t f", j=4),
                        in1=qg[:, None, :, :].to_broadcast([P, 4, 4, E]),
                        op=AL.mult)
                    sc = scp.tile([P, 16], dt.float32, tag="sc")
                    nc.vector.tensor_reduce(out=sc[:], in_=prod[:],
                                            axis=mybir.AxisListType.X,
                                            op=AL.add)
                    ex = scp.tile([P, 16], dt.float32, tag="ex")
                    nc.scalar.activation(out=ex[:], in_=sc[:],
                                         func=mybir.ActivationFunctionType.Exp)
                    den = scp.tile([P, 4], dt.float32, tag="den")
                    nc.vector.tensor_reduce(
                        out=den[:],
                        in_=ex[:].rearrange("p (j t) -> p t j", j=4),
                        axis=mybir.AxisListType.X, op=AL.add)
                    rec = scp.tile([P, 4], dt.float32, tag="rec")
                    nc.vector.reciprocal(out=rec[:], in_=den[:])
                    attn = scp.tile([P, 16], dt.bfloat16, tag="attn")
                    nc.vector.tensor_tensor(
                        out=attn[:].rearrange("p (j t) -> p j t", j=4),
                        in0=ex[:].rearrange("p (j t) -> p j t", j=4),
                        in1=rec[:, None, :].to_broadcast([P, 4, 4]),
                        op=AL.mult)
                    xb = xbar[:, g * 4:(g + 1) * 4, :]
                    nc.vector.tensor_tensor(
                        out=xb, in0=x[:, 0:4, 0:CV],
                        in1=attn[:, 0:4, None].to_broadcast([P, 4, CV]),
                        op=AL.mult)
                    tmp = scp.tile([P, 4, CV], dt.bfloat16, tag="tmp")
                    for j in (1, 2, 3):
                        nc.vector.tensor_tensor(
                            out=tmp[:], in0=x[:, j * 4:(j + 1) * 4, 0:CV],
                            in1=attn[:, j * 4:j * 4 + 4, None]
                                .to_broadcast([P, 4, CV]),
                            op=AL.mult)
                        nc.vector.tensor_tensor(out=xb, in0=xb, in1=tmp[:],
                                                op=AL.add)

            # ---------- scatter ----------
            wi = 0
            for ct in range(CT):
                t1 = psC.tile([CO, WCT], dt.float32, tag="psC")
                nw = int(nwin_s[ct])
                for wv in range(nw):
                    ch = min(int(ch_lo_s[ct]) + wv, NCHUNKS - 1)
                    mat = small.tile([P, WCT], dt.bfloat16, tag="mat")
                    nc.vector.tensor_tensor(
                        out=mat[:],
                        in0=cellcols[:, wi:wi + 1].to_broadcast([P, WCT]),
                        in1=iota256[:],
                        op=AL.is_equal)
                    nc.tensor.matmul(out=t1[:], lhsT=xbar[:, ch, :],
                                     rhs=mat[:], start=(wv == 0),
                                     stop=(wv == nw - 1))
                    wi += 1
                t1s = small.tile([CO, WCT], dt.bfloat16, tag="t1s")
                nc.scalar.copy(out=t1s[:], in_=t1[:])
                ot = psD.tile([CO, WCT], dt.float32, tag="psD")
                nc.tensor.matmul(out=ot[:], lhsT=wovT[:], rhs=t1s[:],
                                 start=True, stop=True)
                osb = small.tile([CO, WCT], dt.float32, tag="osb")
                nc.scalar.copy(out=osb[:], in_=ot[:])
                nc.sync.dma_start(out=out_d[:, ct * WCT:(ct + 1) * WCT],
                                  in_=osb[:])
    nc.compile()
    return nc


def _install_ntff_shim():
    try:
        import antenv.axon_hooks  # noqa
        return
    except ImportError:
        pass
    try:
        from trn_agent_boot.trn_boot import _ntff_profile_via_ctypes
        hook = _ntff_profile_via_ctypes('/opt/axon/libaxon_pjrt.so')
        mod = types.ModuleType("antenv.axon_hooks")
        mod.get_axon_ntff_profile_hook = lambda: hook
        mod.set_axon_ntff_profile_hook = lambda h: None
        import antenv
        antenv.axon_hooks = mod
        sys.modules["antenv.axon_hooks"] = mod
    except Exception:
        pass


def kernel(**inputs):
    v_feat = np.asarray(inputs["v_feat"], np.float32)
    r_feat = np.asarray(inputs["r_feat"], np.float32)
    Wq = np.asarray(inputs["Wq"], np.float32)
    Wk = np.asarray(inputs["Wk"], np.float32)
    Wv = np.asarray(inputs["Wv"], np.float32)
    Wo = np.asarray(inputs["Wo"], np.float32)
    v2p = np.asarray(inputs["v2p_ind"])
    r2p = np.asarray(inputs["r2p_ind"])
    Mv = v_feat.shape[2]
    Mr = r_feat.shape[2]

    plan = _plan(v2p, r2p)
    nc = _build(plan, Mv)

    A16 = (Wq.T @ Wk / np.sqrt(np.float32(E))).astype(BF16)
    WovT16 = np.ascontiguousarray((Wo @ Wv).T).astype(BF16)

    in_maps = []
    for c in plan["cores"]:
        arr = _core_arrays(c, plan, v_feat, r_feat)
        arr["a16"] = A16
        arr["wovT"] = WovT16
        in_maps.append(arr)

    from concourse.bass_utils import run_bass_kernel_spmd
    _install_ntff_shim()
    trace = bool(inputs.get("_trace", False))
    res = run_bass_kernel_spmd(nc, in_maps, core_ids=list(range(8)),
                               trace=trace)
    out = np.zeros((B, CO, Mr), np.float32)
    for ci, c in enumerate(plan["cores"]):
        o = res.results[ci]["out"]
        w = min(c["width"], plan["W_OUT"])
        out[c["b"], :, c["clo"]:c["clo"] + w] = o[:, :w]
    kernel.last_exec_time_ns = res.exec_time_ns
    return out


kernel.last_exec_time_ns = None
